# revision 3
# baseline (speedup 1.0000x reference)
"""CrossNetMix (MoE cross-network) Trainium2 kernel.

Math per layer (reference):
    gates = softmax(x_l @ gate_w.T)                  # [B, E]
    v     = tanh(x_l @ V[l])                         # [B, E, R]  (per expert)
    v2    = tanh(v @ C[l].T)                         # [B, E, R]  (per expert)
    uv    = v2 @ U[l].T                              # [B, E, D]  (per expert)
    x_l   = x0 * (sum_e gates_e * uv_e + bias[l]) + x_l

Kernel strategy (per core, batch data-parallel over 8 cores):
  - activations kept feature-major ("transposed", [feature, batch]) in SBUF
    so every matmul contracts along the partition dim with weights stationary
  - gate softmax: exp on ACT, sum/replication via tiny PE matmuls with
    ones/one-hot matrices, so gates fold into v2 rows before the U matmul
    (softmax weights sum to 1, so bias passes straight through the mix)
  - float32r everywhere on the PE: full bf16-rate with ~1.5e-4 matmul rel-err
  - input/output layout change via PE transposes (fp32 DMA transpose is not
    supported by the xbar)
"""

import sys

for _p in ("/opt/trn_rl_repo", "/root/.axon_site/_ro/trn_rl_repo"):
    if _p not in sys.path:
        sys.path.insert(0, _p)

import numpy as np
from contextlib import ExitStack

import concourse.bass as bass
import concourse.tile as tile
import concourse.mybir as mybir
from concourse.bass_utils import run_bass_kernel_spmd

B, D, R, E, L = 16384, 1024, 64, 4, 3
N_CORES = 8
BC = B // N_CORES          # 2048 rows per core
CH = 512                   # batch columns per processing chunk
ER = E * R                 # 256

f32 = mybir.dt.float32
f32r = mybir.dt.float32r
AF = mybir.ActivationFunctionType

MAX_WAITS = 1


def split_sync_waits(nc, max_waits=MAX_WAITS):
    """Walrus in this container rejects >1 sync-wait per instruction; spread
    extra waits onto preceding same-engine NoOps."""
    ctr = 0
    for f in nc.m.functions:
        for blk in f.blocks:
            insts = list(blk.instructions)
            new = []
            for inst in insts:
                si = inst.sync_info
                if si is not None and len(si.on_wait) > max_waits:
                    waits = list(si.on_wait)
                    over = waits[:-max_waits]
                    keep = waits[-max_waits:]
                    for i in range(0, len(over), max_waits):
                        chunk = over[i:i + max_waits]
                        nop = mybir.InstNoOp(
                            name=f"waitsplit_{ctr}",
                            sync_info=mybir.SyncInfo(on_wait=chunk, on_update=[]),
                            bass_nofuse=True,
                            engine=inst.engine,
                        )
                        ctr += 1
                        new.append(nop)
                    si.on_wait = keep
                    inst.sync_info = si
                new.append(inst)
            blk.instructions = new
    return ctr


def build_nc(bc=BC, ch=CH):
    assert bc % ch == 0 and ch % 128 == 0
    n_chunks = bc // ch
    jb = ch // 128              # 128-row blocks per chunk
    nsl = ch // 512 if ch >= 512 else 1   # matmul N slices per chunk
    nw = min(ch, 512)           # matmul N width

    nc = bass.Bass("TRN2", target_bir_lowering=False, debug=False)

    in_ap = nc.dram_tensor("inputs", [bc, D], f32r, kind="ExternalInput").ap()
    vcat_ap = nc.dram_tensor("vcat", [L, D, ER], f32r, kind="ExternalInput").ap()
    ucat_ap = nc.dram_tensor("ucat", [L, ER, D], f32r, kind="ExternalInput").ap()
    cbd_ap = nc.dram_tensor("cbd", [L, 2, 128, 128], f32r, kind="ExternalInput").ap()
    gt_ap = nc.dram_tensor("gt", [D, E], f32r, kind="ExternalInput").ap()
    oneh_ap = nc.dram_tensor("oneh", [E, ER], f32r, kind="ExternalInput").ap()
    ones41_ap = nc.dram_tensor("ones41", [E, 1], f32r, kind="ExternalInput").ap()
    ones14_ap = nc.dram_tensor("ones14", [1, E], f32r, kind="ExternalInput").ap()
    ident_ap = nc.dram_tensor("ident", [128, 128], f32r, kind="ExternalInput").ap()
    bias_ap = nc.dram_tensor("bias", [L, D], f32, kind="ExternalInput").ap()
    out_ap = nc.dram_tensor("out", [bc, D], f32, kind="ExternalOutput").ap()

    with tile.TileContext(nc) as tc, ExitStack() as ctx:
        const = ctx.enter_context(tc.tile_pool(name="const", bufs=1))
        state = ctx.enter_context(tc.tile_pool(name="state", bufs=1))
        xin_p = ctx.enter_context(tc.tile_pool(name="xin", bufs=2 * jb))
        v_p = ctx.enter_context(tc.tile_pool(name="vp", bufs=2))
        v2g_p = ctx.enter_context(tc.tile_pool(name="v2gp", bufs=3))
        g4_p = ctx.enter_context(tc.tile_pool(name="g4p", bufs=2))
        wb_p = ctx.enter_context(tc.tile_pool(name="wbp", bufs=3))
        t_p = ctx.enter_context(tc.tile_pool(name="tp", bufs=3))
        ost_p = ctx.enter_context(tc.tile_pool(name="ostp", bufs=3))
        mm_ps = ctx.enter_context(tc.tile_pool(name="mmps", bufs=3, space="PSUM"))
        grep_ps = ctx.enter_context(tc.tile_pool(name="grepps", bufs=2, space="PSUM"))
        sm_ps = ctx.enter_context(tc.tile_pool(name="smps", bufs=2, space="PSUM"))

        # ---- constants into SBUF
        vcat_sb = const.tile([128, L * 8 * ER], f32r)
        for l in range(L):
            for k in range(8):
                nc.sync.dma_start(
                    vcat_sb[:, (l * 8 + k) * ER:(l * 8 + k + 1) * ER],
                    vcat_ap[l, k * 128:(k + 1) * 128, :])
        ucat_sb = const.tile([128, L * 2 * D], f32r)
        for l in range(L):
            for kt in range(2):
                nc.sync.dma_start(
                    ucat_sb[:, (l * 2 + kt) * D:(l * 2 + kt + 1) * D],
                    ucat_ap[l, kt * 128:(kt + 1) * 128, :])
        cbd_sb = const.tile([128, L * 2 * 128], f32r)
        for l in range(L):
            for p in range(2):
                nc.sync.dma_start(
                    cbd_sb[:, (l * 2 + p) * 128:(l * 2 + p + 1) * 128],
                    cbd_ap[l, p])
        gt_sb = const.tile([128, 8 * E], f32r)
        for k in range(8):
            nc.sync.dma_start(gt_sb[:, k * E:(k + 1) * E],
                              gt_ap[k * 128:(k + 1) * 128, :])
        oneh_sb = const.tile([E, ER], f32r)
        nc.sync.dma_start(oneh_sb[:], oneh_ap[:])
        ones41_sb = const.tile([E, 1], f32r)
        nc.sync.dma_start(ones41_sb[:], ones41_ap[:])
        ones14_sb = const.tile([1, E], f32r)
        nc.sync.dma_start(ones14_sb[:], ones14_ap[:])
        ident_sb = const.tile([128, 128], f32r)
        nc.sync.dma_start(ident_sb[:], ident_ap[:])
        bias_sb = const.tile([128, L * 8], f32)
        for l in range(L):
            for m in range(8):
                nc.sync.dma_start(
                    bias_sb[:, l * 8 + m:l * 8 + m + 1],
                    bias_ap[l, m * 128:(m + 1) * 128].unsqueeze(1))

        xT = state.tile([128, 8 * ch], f32r)    # x_l, feature-major
        x0T = state.tile([128, 8 * ch], f32r)   # x_0, feature-major

        for c in range(n_chunks):
            # ---- stage in: DMA natural tiles, PE-transpose into xT, copy x0T
            xins = []
            for j in range(jb):
                xin = xin_p.tile([128, D], f32r)
                nc.sync.dma_start(
                    xin[:], in_ap[c * ch + j * 128:c * ch + (j + 1) * 128, :])
                xins.append(xin)
            for m in range(8):
                pt = mm_ps.tile([128, nw], f32r, tag="mm")
                for j in range(jb):
                    nc.tensor.matmul(
                        pt[:, j * 128:(j + 1) * 128],
                        xins[j][:, m * 128:(m + 1) * 128],
                        ident_sb[:],
                        is_transpose=True,
                        start=(j == 0), stop=(j == jb - 1))
                nc.scalar.activation(xT[:, m * ch:m * ch + nw], pt[:], AF.Copy)
                nc.vector.tensor_copy(x0T[:, m * ch:m * ch + nw],
                                      xT[:, m * ch:m * ch + nw])

            for l in range(L):
                for ns in range(nsl):
                    s0 = ns * 512
                    xsl = lambda k: xT[:, k * ch + s0:k * ch + s0 + nw]
                    # ---- gates: logits -> exp -> sum -> recip -> normalized
                    lp = sm_ps.tile([E, nw], f32, tag="sm")
                    for k in range(8):
                        nc.tensor.matmul(lp[:], gt_sb[:, k * E:(k + 1) * E],
                                         xsl(k), start=(k == 0), stop=(k == 7))
                    exp4 = g4_p.tile([E, nw], f32r, tag="exp4")
                    nc.scalar.activation(exp4[:], lp[:], AF.Exp)
                    sp_ = sm_ps.tile([1, nw], f32, tag="sm")
                    nc.tensor.matmul(sp_[:], ones41_sb[:], exp4[:])
                    rec = g4_p.tile([1, nw], f32r, tag="rec")
                    with nc.allow_low_precision(reason="float32r is 4-byte"):
                        nc.vector.reciprocal(rec[:], sp_[:])
                    r4 = sm_ps.tile([E, nw], f32, tag="sm")
                    nc.tensor.matmul(r4[:], ones14_sb[:], rec[:])
                    g4 = g4_p.tile([E, nw], f32r, tag="g4")
                    nc.vector.tensor_mul(g4[:], exp4[:], r4[:])
                    # replicate each gate across its expert's 64 rows (PSUM-resident)
                    greps = []
                    for mt in range(2):
                        gp = grep_ps.tile([128, nw], f32, tag="grep")
                        nc.tensor.matmul(gp[:], oneh_sb[:, mt * 128:(mt + 1) * 128],
                                         g4[:])
                        greps.append(gp)
                    # ---- V: v = tanh(Vcat.T @ x)
                    vts = []
                    for mt in range(2):
                        vp_ = mm_ps.tile([128, nw], f32, tag="mm")
                        for k in range(8):
                            nc.tensor.matmul(
                                vp_[:],
                                vcat_sb[:, (l * 8 + k) * ER + mt * 128:
                                        (l * 8 + k) * ER + (mt + 1) * 128],
                                xsl(k), start=(k == 0), stop=(k == 7))
                        vt = v_p.tile([128, nw], f32r, tag="v")
                        nc.scalar.activation(vt[:], vp_[:], AF.Tanh)
                        vts.append(vt)
                    # ---- C: v2 = tanh(Cbd.T @ v), then gate-weight it
                    v2gs = []
                    for mt in range(2):
                        cp = mm_ps.tile([128, nw], f32, tag="mm")
                        nc.tensor.matmul(
                            cp[:], cbd_sb[:, (l * 2 + mt) * 128:(l * 2 + mt + 1) * 128],
                            vts[mt][:])
                        v2 = v_p.tile([128, nw], f32r, tag="v2")
                        nc.scalar.activation(v2[:], cp[:], AF.Tanh)
                        v2g = v2g_p.tile([128, nw], f32r, tag="v2g")
                        nc.vector.tensor_mul(v2g[:], v2[:], greps[mt][:])
                        v2gs.append(v2g)
                    # ---- U: W = Ucat.T @ v2g ; x += x0 * (W + bias)
                    for m in range(8):
                        wp = mm_ps.tile([128, nw], f32, tag="mm")
                        for kt in range(2):
                            nc.tensor.matmul(
                                wp[:],
                                ucat_sb[:, (l * 2 + kt) * D + m * 128:
                                        (l * 2 + kt) * D + (m + 1) * 128],
                                v2gs[kt][:], start=(kt == 0), stop=(kt == 1))
                        wb = wb_p.tile([128, nw], f32r, tag="wb")
                        nc.scalar.activation(wb[:], wp[:], AF.Identity,
                                             bias=bias_sb[:, l * 8 + m:l * 8 + m + 1])
                        t = t_p.tile([128, nw], f32r, tag="t")
                        nc.vector.tensor_mul(
                            t[:], wb[:], x0T[:, m * ch + s0:m * ch + s0 + nw])
                        nc.vector.tensor_add(
                            xT[:, m * ch + s0:m * ch + s0 + nw],
                            xT[:, m * ch + s0:m * ch + s0 + nw], t[:])

            # ---- stage out: PE-transpose back to natural, DMA to DRAM
            for j in range(jb):
                for half in range(2):
                    op_ = mm_ps.tile([128, nw], f32r, tag="mm")
                    for mi in range(4):
                        m = half * 4 + mi
                        nc.tensor.matmul(
                            op_[:, mi * 128:(mi + 1) * 128],
                            xT[:, m * ch + j * 128:m * ch + (j + 1) * 128],
                            ident_sb[:],
                            is_transpose=True,
                            start=(mi == 0), stop=(mi == 3))
                    ost = ost_p.tile([128, nw], f32, tag="ost")
                    nc.scalar.activation(ost[:], op_[:], AF.Copy)
                    nc.sync.dma_start(
                        out_ap[c * ch + j * 128:c * ch + (j + 1) * 128,
                               half * 512:half * 512 + nw],
                        ost[:])

    return nc


def prep_weights(U, V, C, gate_w, bias):
    U = np.asarray(U, dtype=np.float32)
    V = np.asarray(V, dtype=np.float32)
    C = np.asarray(C, dtype=np.float32)
    gate_w = np.asarray(gate_w, dtype=np.float32)
    bias = np.asarray(bias, dtype=np.float32)
    vcat = np.ascontiguousarray(V.transpose(0, 2, 1, 3).reshape(L, D, ER))
    ucat = np.ascontiguousarray(U.transpose(0, 1, 3, 2).reshape(L, ER, D))
    cbd = np.zeros((L, 2, 128, 128), dtype=np.float32)
    for l in range(L):
        for p in range(2):
            cbd[l, p, :R, :R] = C[l, 2 * p].T
            cbd[l, p, R:, R:] = C[l, 2 * p + 1].T
    gt = np.ascontiguousarray(gate_w.T)
    oneh = np.kron(np.eye(E), np.ones((1, R))).astype(np.float32)
    ones41 = np.ones((E, 1), dtype=np.float32)
    ones14 = np.ones((1, E), dtype=np.float32)
    ident = np.eye(128, dtype=np.float32)
    return dict(vcat=vcat, ucat=ucat, cbd=cbd, gt=gt, oneh=oneh,
                ones41=ones41, ones14=ones14, ident=ident, bias=bias)


_NC_CACHE = {}


def get_nc(bc=BC, ch=CH):
    key = (bc, ch)
    if key not in _NC_CACHE:
        nc = build_nc(bc, ch)
        split_sync_waits(nc)
        _NC_CACHE[key] = nc
    return _NC_CACHE[key]


def make_in_maps(inputs, U, V, C, gate_w, bias):
    inputs = np.ascontiguousarray(np.asarray(inputs, dtype=np.float32))
    w = prep_weights(U, V, C, gate_w, bias)
    in_maps = []
    for c in range(N_CORES):
        m = {"inputs": inputs[c * BC:(c + 1) * BC]}
        m.update(w)
        in_maps.append(m)
    return in_maps


def kernel(inputs, U, V, C, gate_w, bias):
    nc = get_nc()
    in_maps = make_in_maps(inputs, U, V, C, gate_w, bias)
    res = run_bass_kernel_spmd(nc, in_maps, list(range(N_CORES)))
    out = np.concatenate([res.results[c]["out"] for c in range(N_CORES)], axis=0)
    return out.astype(np.float32)
